# revision 27
# baseline (speedup 1.0000x reference)
"""Trainium2 Bass kernel for nn_AttentionBlock (B=16, C=512, H=W=32, 8 heads, d_k=64).

Sharding: data-parallel over batch; each of the 8 NeuronCores computes 2 batches.

Layout is fully transposed (channels on partitions) so no transposes are needed.
The attention phase is ACT(exp)-bound (16.8M exps/core @ 1 elem/lane/cycle,
1.2 GHz); everything else is scheduled into the PE/DVE slack under the exp
stream:

  qkT projection  : fp16 matmuls; qpair[p] = [q_{2p}; q_{2p+1}]^T stacked on
                    partition halves, kp[p] likewise (no zero padding).
  scores          : two K=64 matmuls per step (rows 0-63 / 64-127).
  exp             : ACT -> fp8-e5m2, shift -2.5 keeps exp in e5m2 range;
                    softmax-invariant. Output kappa-interleaved in
                    expbuf[128, 2, 1024] so pairs of j-tiles feed DoubleRow.
  attnv           : fp8 DoubleRow matmuls (2x): [1|0pad|v] e4m3 stationary
                    [128,2,128] x exp e5m2 moving [128,2,512] accumulating
                    res[128, 1024] over 4 j-tile pairs; row 0 = sumexp
                    (reciprocal_approx_fast needs base partition 0), v-dims
                    at rows 64-127 (64-partition ops need base 0/64).
  norm            : res evacuated PSUM->SBUF in one copy (frees the banks for
                    the next group), reciprocal_approx_fast, DMA partition-
                    broadcast via DRAM, 2 DVE mults -> res_pair e4m3.
  v projection    : fp8 DoubleRow matmuls (2x) over pair-interleaved
                    x8/wv8 (host-prepared).
  out projection  : fp16 matmuls (fp8 here costs too much accuracy) +
                    residual (x + b_out prefused, fp16); y stored fp16.

Schedule: one flat 128-step exp stream (16 groups of (batch, head-pair,
query-half) x 8 j-tiles); projection/output-projection/DMA work is interleaved
as PE filler so ACT never starves.
"""
from collections import deque

import numpy as np
import ml_dtypes

import concourse.bass as bass
from concourse import bacc
import concourse.mybir as mybir
import concourse.tile as tile
from concourse import bass_utils

F32 = mybir.dt.float32
F16 = mybir.dt.float16
F8E4 = mybir.dt.float8e4
F8E5 = mybir.dt.float8e5
AF = mybir.ActivationFunctionType
ALU = mybir.AluOpType
DR = mybir.MatmulPerfMode.DoubleRow

N_HEADS = 8
DK = 64
SCALE = DK ** -0.5
EXP_SHIFT = -2.5       # max scaled logit ~12.2 -> max exp arg ~9.7, e^9.7=16k
C = 512                # < e5m2 max 57344; per-query max weight >= 0.46
N = 1024               # tokens per batch (32*32)
NB = 2                 # batches per core
NCORES = 8
NCH = C // 128         # 4 contraction chunks
NCP = NCH // 2         # 2 DoubleRow chunk-pairs
NT = N // 128          # 8 token tiles
NU = NT // 2           # 4 j-tile pairs for DoubleRow attnv
NPAIR = N_HEADS // 2
VW = 128               # per-head v_aug extent (1 ones + 63 pad + 64 v dims)
DEBUG = False


def build():
    nc = bacc.Bacc(None, target_bir_lowering=False, num_swdge_queues=4)
    dbg = {}
    if DEBUG:
        dbg["q0"] = nc.dram_tensor("dbg_q0", (128, N), F16, kind="ExternalOutput")
        dbg["k0"] = nc.dram_tensor("dbg_k0", (128, N), F16, kind="ExternalOutput")
        dbg["exp0"] = nc.dram_tensor("dbg_exp0", (128, 2, N), F8E5, kind="ExternalOutput")
        dbg["vaug"] = nc.dram_tensor("dbg_vaug", (128, NU, 2, N_HEADS, VW), F8E4, kind="ExternalOutput")
        dbg["ressb"] = nc.dram_tensor("dbg_ressb", (128, N), F32, kind="ExternalOutput")
        dbg["rcp"] = nc.dram_tensor("dbg_rcp", (1, N), F32, kind="ExternalOutput")
        dbg["respair"] = nc.dram_tensor("dbg_respair", (NPAIR, 128, N), F16, kind="ExternalOutput")
    x_d = nc.dram_tensor("x", (NB, C, N), F16, kind="ExternalInput")
    x8_d = nc.dram_tensor("x8", (NB, NCP, 128, 2, N), F8E4, kind="ExternalInput")
    xpb_d = nc.dram_tensor("xpb", (NB, C, N), F16, kind="ExternalInput")
    wqk_d = nc.dram_tensor("w_qk", (C, 2, NPAIR, 128), F16, kind="ExternalInput")
    bqkt_d = nc.dram_tensor("b_qk_t", (128, 2, NPAIR), F32, kind="ExternalInput")
    wv_d = nc.dram_tensor("w_v8", (NCP, 128, 2, C), F8E4, kind="ExternalInput")
    bv_d = nc.dram_tensor("b_v", (1, C), F32, kind="ExternalInput")
    wout_d = nc.dram_tensor("w_out", (C, C), F16, kind="ExternalInput")
    z8_d = nc.dram_tensor("z8", (1, NU, 2, N_HEADS, VW), F8E4,
                          kind="ExternalInput")
    y_d = nc.dram_tensor("y", (NB, C, N), F16, kind="ExternalOutput")

    with tile.TileContext(nc) as tc:
        with (
            tc.tile_pool(name="const", bufs=1) as const,
            tc.tile_pool(name="persist", bufs=1) as persist,
            tc.tile_pool(name="sbwork", bufs=3) as sbwork,
            tc.tile_pool(name="sbexp", bufs=3) as sbexp,
            tc.tile_pool(name="ps_s", bufs=2, space="PSUM") as ps_s,
            tc.tile_pool(name="ps_acc", bufs=2, space="PSUM") as ps_acc,
            tc.tile_pool(name="dram", bufs=8, space="DRAM") as dram,
        ):
            # ---- weights / x loads (fp16 + fp8 pair-interleaved) ----
            x_r = [persist.tile([128, N], F16, name=f"xr{ch}")
                   for ch in range(NCH)]
            x8_r = [persist.tile([128, 2, N], F8E4, name=f"x8r{c}")
                    for c in range(NCP)]
            # v_aug zero-fill via DMA broadcast (a DVE/Pool memset of 1MB
            # would hog an engine for ~7us right when the head needs it)
            v_aug = persist.tile([128, NU, 2, N_HEADS, VW], F8E4)
            nc.sync.dma_start(
                v_aug[:],
                z8_d[:].to_broadcast([128, NU, 2, N_HEADS, VW]))
            # x on the gpsimd SWDGE queues (4-way round-robin, no HWDGE
            # cross-queue ordering chains), qk weights on sync/scalar HWDGE
            # — mixing them serialized the whole head via queue tokens.
            wqk = []
            weng = [nc.sync, nc.sync, nc.scalar, nc.scalar]
            for ch in range(NCH):
                nc.gpsimd.dma_start(x_r[ch][:],
                                    x_d[0, ch * 128:(ch + 1) * 128, :])
            for ch in range(NCH):
                w = const.tile([128, 2, NPAIR, 128], F16, name=f"wqk{ch}")
                weng[ch].dma_start(w[:], wqk_d[ch * 128:(ch + 1) * 128])
                wqk.append(w)
            for c in range(NCP):
                nc.gpsimd.dma_start(x8_r[c][:], x8_d[0, c])
            wv8 = []
            for c in range(NCP):
                w2 = const.tile([128, 2, C], F8E4, name=f"wv8{c}")
                nc.gpsimd.dma_start(w2[:], wv_d[c])
                wv8.append(w2)
            bqkt = const.tile([128, 2, NPAIR], F32)
            nc.sync.dma_start(bqkt[:], bqkt_d[:])
            bv_bc = const.tile([128, C], F32)   # b_v broadcast to all partitions
            nc.sync.dma_start(bv_bc[:], bv_d[:].to_broadcast([128, C]))

            # HAM warmup: dummy matmuls on memset data run during the initial
            # DMA wait so the real matmuls start at the full PE clock.
            warm = const.tile([128, 512], F16)
            nc.vector.memset(warm[:], 0.5)
            warm_ps = ps_acc.tile([128, 512], F32, tag="acc", name="warm_ps")
            for r in range(10):
                nc.tensor.matmul(warm_ps[:], warm[:, 0:128], warm[:],
                                 start=(r == 0), stop=(r == 9))
            # preload the exp table set during the DMA head (first real exp
            # would otherwise pay the ~2.7us ACT_TABLE_LOAD inside the stream)
            warm_exp = const.tile([128, 8], F16)
            nc.scalar.activation(out=warm_exp[:], in_=warm[:, 0:8],
                                 func=AF.Exp, scale=1.0)
            expbias = const.tile([128, 1], F32)
            nc.vector.memset(expbias[:], EXP_SHIFT)

            # ---- persistent per-batch buffers ----
            qpair = [persist.tile([128, N], F16, name=f"qpair{p}")
                     for p in range(NPAIR)]
            kp = [persist.tile([128, N], F16, name=f"kp{p}")
                  for p in range(NPAIR)]
            # ones row of v_aug (sumexp): [j, u, kappa, h, 0] = 1
            nc.vector.memset(v_aug[:, :, :, :, 0:1], 1.0)
            res_all_db = [[persist.tile([128, N], F16, name=f"resall{bb}_{p}")
                           for p in range(NPAIR)] for bb in range(NB)]

            # ---- work units (closures) for PE-filler interleaving ----
            xeng_mid = [nc.sync, nc.gpsimd, nc.gpsimd, nc.sync]

            def xload_unit(b, ch):
                def f():
                    # mid-stream: keep DMA issue off the busy Scalar queue
                    xeng_mid[ch].dma_start(x_r[ch][:],
                                           x_d[b, ch * 128:(ch + 1) * 128, :])
                    if ch % 2 == 0:
                        xeng_mid[ch + 1].dma_start(x8_r[ch // 2][:],
                                                   x8_d[b, ch // 2])
                return f

            def qkT_unit(p, qk, nh):
                def f():
                    nsl = slice(nh * 512, nh * 512 + 512)
                    ps = ps_acc.tile([128, 512], F32, tag="acc", name="qk_ps")
                    for ch in range(NCH):
                        nc.tensor.matmul(
                            ps[:], wqk[ch][:, qk, p, :], x_r[ch][:, nsl],
                            start=(ch == 0), stop=(ch == NCH - 1))
                    dst = qpair[p] if qk == 0 else kp[p]
                    nc.vector.tensor_scalar(
                        out=dst[:, nsl], in0=ps[:],
                        scalar1=bqkt[:, qk, p:p + 1], scalar2=None,
                        op0=ALU.add)
                return f

            def v_unit(t):
                def f():
                    ps = ps_acc.tile([128, 512], F32, tag="acc", name="v_ps")
                    for c in range(NCP):
                        nc.tensor.matmul(
                            ps[:], x8_r[c][:, :, t * 128:(t + 1) * 128],
                            wv8[c][:], perf_mode=DR,
                            start=(c == 0), stop=(c == NCP - 1))
                    nc.vector.tensor_add(
                        v_aug[:, t // 2, t % 2, :, 64:64 + DK],
                        ps[:].rearrange("p (h d) -> p h d", h=N_HEADS),
                        bv_bc[:].rearrange("p (h d) -> p h d", h=N_HEADS))
                return f

            def out_units(b):
                xres = {}
                units = []

                def mk(ct, nh):
                    def f():
                        csl = slice(ct * 128, (ct + 1) * 128)
                        nsl = slice(nh * 512, nh * 512 + 512)
                        if ct not in xres:
                            xr = sbwork.tile([128, N], F16, tag="xres", bufs=4,
                                             name=f"x_res{b}_{ct}")
                            nc.sync.dma_start(xr[:], xpb_d[b, csl, :])
                            xres[ct] = xr
                        ps = ps_acc.tile([128, 512], F32, tag="acc",
                                         name="out_ps")
                        for ch in range(NCH):
                            nc.tensor.matmul(
                                ps[:], wo[ch][:, csl],
                                res_all_db[b][ch][:, nsl],
                                start=(ch == 0), stop=(ch == NCH - 1))
                        out_sb = sbwork.tile([128, 512], F16, tag="out",
                                             name="out_sb")
                        nc.vector.tensor_add(out_sb[:], ps[:],
                                             xres[ct][:, nsl])
                        nc.sync.dma_start(y_d[b, csl, nsl], out_sb[:])
                    return f

                for nh in range(2):
                    for ct in range(NCH):
                        units.append(mk(ct, nh))
                return units[:NCH], units[NCH:]

            filler = deque()

            def inject(k=1):
                for _ in range(min(k, len(filler))):
                    filler.popleft()()

            def norm_group(b, p, ic, res_ps):
                # res_ps [128, 1024]: head 2p cols 0-511, head 2p+1 cols
                # 512-1023; row 0 = sumexp, rows 64-127 = v dims.  One
                # full-tile copy evacuates PSUM immediately (DVE cost is
                # free-dim-driven) — releases the res banks for the next
                # group ~4us earlier than normalizing out of PSUM would.
                # (Also: reciprocal_approx_fast reads garbage from PSUM.)
                isl = slice(ic * 512, ic * 512 + 512)
                res_sb = sbwork.tile([128, N], F32, tag="ressb", bufs=2,
                                     name="res_sb")
                nc.vector.tensor_copy(res_sb[:], res_ps[:])
                rcp_sb = sbwork.tile([1, N], F32, tag="sums", name="rcp_sb")
                nc.vector.reciprocal_approx_fast(out=rcp_sb[:],
                                                 in_=res_sb[0:1, :])
                rcp_dram = dram.tile([1, N], F32, tag="sumd", name="rcp_dram")
                nc.gpsimd.dma_start(rcp_dram[:], rcp_sb[:])
                # broadcast into rows 64-127 only, so the mult's two SBUF
                # inputs share base partition 64 (NCC_IBIR297)
                mult = sbwork.tile([128, N], F32, tag="mult", name="mult")
                nc.gpsimd.dma_start(mult[64:128, :],
                                    rcp_dram[:].to_broadcast([64, N]))
                for s in range(2):
                    nc.vector.tensor_mul(
                        res_all_db[b][p][s * 64:(s + 1) * 64, isl],
                        res_sb[64:64 + DK, s * 512:(s + 1) * 512],
                        mult[64:128, s * 512:(s + 1) * 512])
                if DEBUG and (b, p, ic) == (0, 0, 0):
                    nc.sync.dma_start(dbg["ressb"][:], res_sb[:])
                    nc.sync.dma_start(dbg["rcp"][:], rcp_sb[:])
                    nc.sync.dma_start(dbg["q0"][:], qpair[0][:])
                    nc.sync.dma_start(dbg["k0"][:], kp[0][:])
                    nc.sync.dma_start(dbg["vaug"][:], v_aug[:])

            # ---- emission schedule: one flat attention stream ----
            wo = []
            wo_units = []
            for ch in range(NCH):
                w = const.tile([128, C], F16, name=f"wout{ch}")
                wo.append(w)

                def mk_wo(ch=ch, w=w):
                    def f():
                        nc.sync.dma_start(w[:],
                                          wout_d[ch * 128:(ch + 1) * 128, :])
                    return f
                wo_units.append(mk_wo())

            def qkts(p):
                # need-order: q/k nh0 halves first (used from t=0 of the
                # pair's first group), nh1 halves after (used from t=4).
                return [qkT_unit(p, qk, nh) for nh in range(2)
                        for qk in range(2)]

            with nc.named_scope("b0_proj"):
                # only the nh=0 halves up front; scores t=0..3 need just these.
                qkT_unit(0, 0, 0)()
                qkT_unit(0, 1, 0)()

            b0_nh0, b0_nh1 = out_units(0)
            b1_nh0, b1_nh1 = out_units(1)
            b0_all = b0_nh0 + b0_nh1

            groups = [(b, p, ic) for b in range(NB) for p in range(NPAIR)
                      for ic in range(2)]
            group_fill = {
                0: [qkT_unit(0, 0, 1), qkT_unit(0, 1, 1)] + qkts(1),
                2: qkts(2), 3: qkts(3),
                4: [xload_unit(1, ch) for ch in range(NCH)],
                5: wo_units,
                6: qkts(0),                      # batch-1 weights from here
                7: qkts(1), 9: qkts(2), 10: qkts(3),
                11: b0_all[0:3], 12: b0_all[3:6], 13: b0_all[6:8],
                15: b1_nh0,
            }
            # v(b1, 2u..2u+1) is legal only after attnv(b0,p3,ic1,u) (WAR on
            # v_aug) and before attnv(b1,p0,ic0,u) (RAW): pop 2 right after
            # group 7's attnv pair emissions.
            group_post = {7: deque(v_unit(t) for t in range(NT))}
            pre = deque(v_unit(t) for t in range(NT))   # batch-0 v, group 0

            nsteps = len(groups) * NT
            pending = deque()
            res_of = {}
            eb_of = {}
            with nc.named_scope("attn_stream"):
                for k in range(nsteps + 4):
                    g, t = divmod(k, NT)
                    if k < nsteps:
                        b, p, ic = groups[g]
                        if t == 0:
                            if g in group_fill:
                                filler.extend(group_fill[g])
                            res_of[g] = ps_acc.tile(
                                [128, N], F32, tag="res", bufs=1,
                                name=f"res{g}")
                        isl = slice(ic * 512, ic * 512 + 512)
                        js = slice(t * 128, (t + 1) * 128)
                        s_ps = ps_s.tile([128, N], F32, tag="big", name="s_ps")
                        nc.tensor.matmul(s_ps[:, 0:512], kp[p][0:64, js],
                                         qpair[p][0:64, isl],
                                         start=True, stop=True)
                        nc.tensor.matmul(s_ps[:, 512:1024], kp[p][64:128, js],
                                         qpair[p][64:128, isl],
                                         start=True, stop=True)
                        if t % 2 == 0:
                            eb_of[g, t // 2] = sbexp.tile(
                                [128, 2, N], F8E5, tag="exp", bufs=3,
                                name="exp_sb")
                        eb = eb_of[g, t // 2]
                        nc.scalar.activation(out=eb[:, t % 2, :], in_=s_ps[:],
                                             func=AF.Exp, bias=expbias[:],
                                             scale=SCALE)
                        if g == 0 and pre:
                            pre.popleft()()
                        if t % 2 == 1:
                            pending.append((g, t // 2))
                        elif t >= 2:
                            # t=0 is excluded: fillers must not be emitted
                            # before the cross-group norm pop at t=1.
                            inject(1)
                    if len(pending) == 2 or (k >= nsteps and pending):
                        pg, pu = pending.popleft()
                        pb, pp, pic = groups[pg]
                        peb = eb_of.pop((pg, pu))
                        if DEBUG and (pg, pu) == (0, 0):
                            nc.sync.dma_start(dbg["exp0"][:], peb[:])
                        for s in range(2):
                            nc.tensor.matmul(
                                res_of[pg][:, s * 512:(s + 1) * 512],
                                v_aug[:, pu, :, 2 * pp + s, :],
                                peb[:, :, s * 512:(s + 1) * 512],
                                perf_mode=DR,
                                start=(pu == 0), stop=(pu == NU - 1))
                        if pu == NU - 1:
                            norm_group(pb, pp, pic, res_of.pop(pg))
                        if pg in group_post and group_post[pg]:
                            group_post[pg].popleft()()
                            group_post[pg].popleft()()
                        else:
                            inject(1)

            with nc.named_scope("b1_out"):
                inject(len(filler))
                for u in b1_nh1:
                    u()
                if DEBUG:
                    for p in range(NPAIR):
                        nc.sync.dma_start(dbg["respair"][p],
                                          res_all_db[0][p][:])

    nc.finalize()
    return nc


_NC = None


def _get_nc():
    global _NC
    if _NC is None:
        _NC = build()
    return _NC


def _to_e4m3(a):
    return np.clip(a, -240, 240).astype(ml_dtypes.float8_e4m3fn)


def make_in_maps(x, W_qkv, b_qkv, W_out, b_out):
    x = np.ascontiguousarray(np.asarray(x, np.float32)).reshape(16, C, N)
    b_out = np.asarray(b_out, np.float32)
    xpb = np.ascontiguousarray(x + b_out[None, :, None]).astype(np.float16)
    w3 = np.asarray(W_qkv, np.float32).reshape(C, N_HEADS, 3, DK)
    w_qk = np.ascontiguousarray(
        np.stack([w3[:, :, 0], w3[:, :, 1]], axis=1).reshape(C, 2, NPAIR, 128))
    w_v = np.ascontiguousarray(w3[:, :, 2].reshape(C, C))
    # pair-interleave the contraction dim for DoubleRow: [cpair, 128, 2, out]
    w_v8 = _to_e4m3(w_v.reshape(NCP, 2, 128, C).transpose(0, 2, 1, 3))
    x8 = _to_e4m3(x.reshape(16, NCP, 2, 128, N).transpose(0, 1, 3, 2, 4))
    b3 = np.asarray(b_qkv, np.float32).reshape(N_HEADS, 3, DK)
    b_qk_t = np.ascontiguousarray(
        np.stack([b3[:, 0], b3[:, 1]], axis=0)
        .reshape(2, NPAIR, 128).transpose(2, 0, 1))
    b_v = np.ascontiguousarray(b3[:, 2].reshape(1, C))
    z8 = np.zeros((1, NU, 2, N_HEADS, VW), dtype=ml_dtypes.float8_e4m3fn)
    maps = []
    for core in range(NCORES):
        maps.append({
            "x": x[core * NB:(core + 1) * NB].astype(np.float16),
            "x8": np.ascontiguousarray(x8[core * NB:(core + 1) * NB]),
            "xpb": xpb[core * NB:(core + 1) * NB],
            "w_qk": w_qk.astype(np.float16),
            "b_qk_t": b_qk_t,
            "w_v8": np.ascontiguousarray(w_v8),
            "b_v": b_v,
            "w_out": np.asarray(W_out, np.float16),
            "z8": z8,
        })
    return maps


def run_on_hw(in_maps, **kwargs):
    nc = _get_nc()
    return bass_utils.run_bass_kernel_spmd(
        nc, in_maps, core_ids=list(range(NCORES)), **kwargs)


def kernel(x, W_qkv, b_qkv, W_out, b_out):
    res = run_on_hw(make_in_maps(x, W_qkv, b_qkv, W_out, b_out))
    y = np.concatenate([r["y"] for r in res.results], axis=0)  # (16, C, N)
    return y.reshape(16, C, 32, 32).astype(np.float32)


# revision 28
# speedup vs baseline: 1.1328x; 1.1328x over previous
"""Trainium2 Bass kernel for nn_AttentionBlock (B=16, C=512, H=W=32, 8 heads, d_k=64).

Sharding: data-parallel over batch; each of the 8 NeuronCores computes 2 batches.

Layout is fully transposed (channels on partitions) so no transposes are needed.
The attention phase is ACT(exp)-bound (16.8M exps/core @ 1 elem/lane/cycle,
1.2 GHz); everything else is scheduled into the PE/DVE slack under the exp
stream:

  qkT projection  : fp16 matmuls; qpair[p] = [q_{2p}; q_{2p+1}]^T stacked on
                    partition halves, kp[p] likewise (no zero padding).
  scores          : two K=64 matmuls per step (rows 0-63 / 64-127).
  exp             : ACT -> fp8-e5m2, shift -2.5 keeps exp in e5m2 range;
                    softmax-invariant. Output kappa-interleaved in
                    expbuf[128, 2, 1024] so pairs of j-tiles feed DoubleRow.
  attnv           : fp8 DoubleRow matmuls (2x): [1|0pad|v] e4m3 stationary
                    [128,2,128] x exp e5m2 moving [128,2,512] accumulating
                    res[128, 1024] over 4 j-tile pairs; row 0 = sumexp
                    (reciprocal_approx_fast needs base partition 0), v-dims
                    at rows 64-127 (64-partition ops need base 0/64).
  norm            : res evacuated PSUM->SBUF in one copy (frees the banks for
                    the next group), reciprocal_approx_fast, DMA partition-
                    broadcast via DRAM, 2 DVE mults -> res_pair e4m3.
  v projection    : fp8 DoubleRow matmuls (2x) over pair-interleaved
                    x8/wv8 (host-prepared).
  out projection  : fp16 matmuls (fp8 here costs too much accuracy) +
                    residual (x + b_out prefused, fp16); y stored fp16.

Schedule: one flat 128-step exp stream (16 groups of (batch, head-pair,
query-half) x 8 j-tiles); projection/output-projection/DMA work is interleaved
as PE filler so ACT never starves.
"""
from collections import deque

import numpy as np
import ml_dtypes

import concourse.bass as bass
from concourse import bacc
import concourse.mybir as mybir
import concourse.tile as tile
from concourse import bass_utils

F32 = mybir.dt.float32
F16 = mybir.dt.float16
F8E4 = mybir.dt.float8e4
F8E5 = mybir.dt.float8e5
AF = mybir.ActivationFunctionType
ALU = mybir.AluOpType
DR = mybir.MatmulPerfMode.DoubleRow

N_HEADS = 8
DK = 64
SCALE = DK ** -0.5
EXP_SHIFT = -2.5       # max scaled logit ~12.2 -> max exp arg ~9.7, e^9.7=16k
C = 512                # < e5m2 max 57344; per-query max weight >= 0.46
N = 1024               # tokens per batch (32*32)
NB = 2                 # batches per core
NCORES = 8
NCH = C // 128         # 4 contraction chunks
NCP = NCH // 2         # 2 DoubleRow chunk-pairs
NT = N // 128          # 8 token tiles
NU = NT // 2           # 4 j-tile pairs for DoubleRow attnv
NPAIR = N_HEADS // 2
VW = 128               # per-head v_aug extent (1 ones + 63 pad + 64 v dims)
DEBUG = False


def build():
    nc = bacc.Bacc(None, target_bir_lowering=False, num_swdge_queues=4)
    dbg = {}
    if DEBUG:
        dbg["q0"] = nc.dram_tensor("dbg_q0", (128, N), F16, kind="ExternalOutput")
        dbg["k0"] = nc.dram_tensor("dbg_k0", (128, N), F16, kind="ExternalOutput")
        dbg["exp0"] = nc.dram_tensor("dbg_exp0", (128, 2, N), F8E5, kind="ExternalOutput")
        dbg["vaug"] = nc.dram_tensor("dbg_vaug", (128, NU, 2, N_HEADS, VW), F8E4, kind="ExternalOutput")
        dbg["ressb"] = nc.dram_tensor("dbg_ressb", (128, N), F32, kind="ExternalOutput")
        dbg["rcp"] = nc.dram_tensor("dbg_rcp", (1, N), F32, kind="ExternalOutput")
        dbg["respair"] = nc.dram_tensor("dbg_respair", (NPAIR, 128, N), F16, kind="ExternalOutput")
    x_d = nc.dram_tensor("x", (NB, C, N), F16, kind="ExternalInput")
    x8_d = nc.dram_tensor("x8", (NB, NCP, 128, 2, N), F8E4, kind="ExternalInput")
    xpb_d = nc.dram_tensor("xpb", (NB, C, N), F16, kind="ExternalInput")
    wqk_d = nc.dram_tensor("w_qk", (C, 2, NPAIR, 128), F16, kind="ExternalInput")
    bqkt_d = nc.dram_tensor("b_qk_t", (128, 2, NPAIR), F32, kind="ExternalInput")
    wv_d = nc.dram_tensor("w_v8", (NCP, 128, 2, C), F8E4, kind="ExternalInput")
    bv_d = nc.dram_tensor("b_v", (1, C), F32, kind="ExternalInput")
    wout_d = nc.dram_tensor("w_out", (C, C), F16, kind="ExternalInput")
    z8_d = nc.dram_tensor("z8", (1, NU, 2, N_HEADS, VW), F8E4,
                          kind="ExternalInput")
    y_d = nc.dram_tensor("y", (NB, C, N), F16, kind="ExternalOutput")

    with tile.TileContext(nc) as tc:
        with (
            tc.tile_pool(name="const", bufs=1) as const,
            tc.tile_pool(name="persist", bufs=1) as persist,
            tc.tile_pool(name="sbwork", bufs=3) as sbwork,
            tc.tile_pool(name="sbexp", bufs=3) as sbexp,
            tc.tile_pool(name="ps_s", bufs=2, space="PSUM") as ps_s,
            tc.tile_pool(name="ps_acc", bufs=2, space="PSUM") as ps_acc,
            tc.tile_pool(name="dram", bufs=8, space="DRAM") as dram,
        ):
            # ---- weights / x loads (fp16 + fp8 pair-interleaved) ----
            x_r = [persist.tile([128, N], F16, name=f"xr{ch}")
                   for ch in range(NCH)]
            x8_r = [persist.tile([128, 2, N], F8E4, name=f"x8r{c}")
                    for c in range(NCP)]
            # v_aug zero-fill via DMA broadcast (a DVE/Pool memset of 1MB
            # would hog an engine for ~7us right when the head needs it)
            v_aug = persist.tile([128, NU, 2, N_HEADS, VW], F8E4)
            nc.scalar.dma_start(
                v_aug[:],
                z8_d[:].to_broadcast([128, NU, 2, N_HEADS, VW]))
            wqk = []
            xeng = [nc.sync, nc.scalar, nc.gpsimd, nc.gpsimd]
            weng = [nc.gpsimd, nc.gpsimd, nc.scalar, nc.sync]
            for ch in range(NCH):
                # x chunks first, spread over the 3 DMA queues (first qkT
                # needs all of them)
                xeng[ch].dma_start(x_r[ch][:], x_d[0, ch * 128:(ch + 1) * 128, :])
            for c in range(NCP):
                [nc.sync, nc.scalar][c].dma_start(x8_r[c][:], x8_d[0, c])
            for ch in range(NCH):
                w = const.tile([128, 2, NPAIR, 128], F16, name=f"wqk{ch}")
                weng[ch].dma_start(w[:], wqk_d[ch * 128:(ch + 1) * 128])
                wqk.append(w)
            wv8 = []
            for c in range(NCP):
                w2 = const.tile([128, 2, C], F8E4, name=f"wv8{c}")
                [nc.gpsimd, nc.sync][c].dma_start(w2[:], wv_d[c])
                wv8.append(w2)
            bqkt = const.tile([128, 2, NPAIR], F32)
            nc.sync.dma_start(bqkt[:], bqkt_d[:])
            bv_bc = const.tile([128, C], F32)   # b_v broadcast to all partitions
            nc.sync.dma_start(bv_bc[:], bv_d[:].to_broadcast([128, C]))

            # HAM warmup: dummy matmuls on memset data run during the initial
            # DMA wait so the real matmuls start at the full PE clock.
            warm = const.tile([128, 512], F16)
            nc.vector.memset(warm[:], 0.5)
            warm_ps = ps_acc.tile([128, 512], F32, tag="acc", name="warm_ps")
            for r in range(10):
                nc.tensor.matmul(warm_ps[:], warm[:, 0:128], warm[:],
                                 start=(r == 0), stop=(r == 9))
            # preload the exp table set during the DMA head (first real exp
            # would otherwise pay the ~2.7us ACT_TABLE_LOAD inside the stream)
            warm_exp = const.tile([128, 8], F16)
            nc.scalar.activation(out=warm_exp[:], in_=warm[:, 0:8],
                                 func=AF.Exp, scale=1.0)
            expbias = const.tile([128, 1], F32)
            nc.vector.memset(expbias[:], EXP_SHIFT)

            # ---- persistent per-batch buffers ----
            qpair = [persist.tile([128, N], F16, name=f"qpair{p}")
                     for p in range(NPAIR)]
            kp = [persist.tile([128, N], F16, name=f"kp{p}")
                  for p in range(NPAIR)]
            # ones row of v_aug (sumexp): [j, u, kappa, h, 0] = 1
            nc.vector.memset(v_aug[:, :, :, :, 0:1], 1.0)
            res_all_db = [[persist.tile([128, N], F16, name=f"resall{bb}_{p}")
                           for p in range(NPAIR)] for bb in range(NB)]

            # ---- work units (closures) for PE-filler interleaving ----
            xeng_mid = [nc.sync, nc.gpsimd, nc.gpsimd, nc.sync]

            def xload_unit(b, ch):
                def f():
                    # mid-stream: keep DMA issue off the busy Scalar queue
                    xeng_mid[ch].dma_start(x_r[ch][:],
                                           x_d[b, ch * 128:(ch + 1) * 128, :])
                    if ch % 2 == 0:
                        xeng_mid[ch + 1].dma_start(x8_r[ch // 2][:],
                                                   x8_d[b, ch // 2])
                return f

            def qkT_unit(p, qk, nh):
                def f():
                    nsl = slice(nh * 512, nh * 512 + 512)
                    ps = ps_acc.tile([128, 512], F32, tag="acc", name="qk_ps")
                    for ch in range(NCH):
                        nc.tensor.matmul(
                            ps[:], wqk[ch][:, qk, p, :], x_r[ch][:, nsl],
                            start=(ch == 0), stop=(ch == NCH - 1))
                    dst = qpair[p] if qk == 0 else kp[p]
                    nc.vector.tensor_scalar(
                        out=dst[:, nsl], in0=ps[:],
                        scalar1=bqkt[:, qk, p:p + 1], scalar2=None,
                        op0=ALU.add)
                return f

            def v_unit(t):
                def f():
                    ps = ps_acc.tile([128, 512], F32, tag="acc", name="v_ps")
                    for c in range(NCP):
                        nc.tensor.matmul(
                            ps[:], x8_r[c][:, :, t * 128:(t + 1) * 128],
                            wv8[c][:], perf_mode=DR,
                            start=(c == 0), stop=(c == NCP - 1))
                    nc.vector.tensor_add(
                        v_aug[:, t // 2, t % 2, :, 64:64 + DK],
                        ps[:].rearrange("p (h d) -> p h d", h=N_HEADS),
                        bv_bc[:].rearrange("p (h d) -> p h d", h=N_HEADS))
                return f

            def out_units(b):
                xres = {}
                units = []

                def mk(ct, nh):
                    def f():
                        csl = slice(ct * 128, (ct + 1) * 128)
                        nsl = slice(nh * 512, nh * 512 + 512)
                        if ct not in xres:
                            xr = sbwork.tile([128, N], F16, tag="xres", bufs=4,
                                             name=f"x_res{b}_{ct}")
                            nc.sync.dma_start(xr[:], xpb_d[b, csl, :])
                            xres[ct] = xr
                        ps = ps_acc.tile([128, 512], F32, tag="acc",
                                         name="out_ps")
                        for ch in range(NCH):
                            nc.tensor.matmul(
                                ps[:], wo[ch][:, csl],
                                res_all_db[b][ch][:, nsl],
                                start=(ch == 0), stop=(ch == NCH - 1))
                        out_sb = sbwork.tile([128, 512], F16, tag="out",
                                             name="out_sb")
                        nc.vector.tensor_add(out_sb[:], ps[:],
                                             xres[ct][:, nsl])
                        nc.sync.dma_start(y_d[b, csl, nsl], out_sb[:])
                    return f

                for nh in range(2):
                    for ct in range(NCH):
                        units.append(mk(ct, nh))
                return units[:NCH], units[NCH:]

            filler = deque()

            def inject(k=1):
                for _ in range(min(k, len(filler))):
                    filler.popleft()()

            def norm_group(b, p, ic, res_ps):
                # res_ps [128, 1024]: head 2p cols 0-511, head 2p+1 cols
                # 512-1023; row 0 = sumexp, rows 64-127 = v dims.  One
                # full-tile copy evacuates PSUM immediately (DVE cost is
                # free-dim-driven) — releases the res banks for the next
                # group ~4us earlier than normalizing out of PSUM would.
                # (Also: reciprocal_approx_fast reads garbage from PSUM.)
                isl = slice(ic * 512, ic * 512 + 512)
                res_sb = sbwork.tile([128, N], F32, tag="ressb", bufs=2,
                                     name="res_sb")
                nc.vector.tensor_copy(res_sb[:], res_ps[:])
                rcp_sb = sbwork.tile([1, N], F32, tag="sums", name="rcp_sb")
                nc.vector.reciprocal_approx_fast(out=rcp_sb[:],
                                                 in_=res_sb[0:1, :])
                rcp_dram = dram.tile([1, N], F32, tag="sumd", name="rcp_dram")
                nc.gpsimd.dma_start(rcp_dram[:], rcp_sb[:])
                # broadcast into rows 64-127 only, so the mult's two SBUF
                # inputs share base partition 64 (NCC_IBIR297)
                mult = sbwork.tile([128, N], F32, tag="mult", name="mult")
                nc.gpsimd.dma_start(mult[64:128, :],
                                    rcp_dram[:].to_broadcast([64, N]))
                for s in range(2):
                    nc.vector.tensor_mul(
                        res_all_db[b][p][s * 64:(s + 1) * 64, isl],
                        res_sb[64:64 + DK, s * 512:(s + 1) * 512],
                        mult[64:128, s * 512:(s + 1) * 512])
                if DEBUG and (b, p, ic) == (0, 0, 0):
                    nc.sync.dma_start(dbg["ressb"][:], res_sb[:])
                    nc.sync.dma_start(dbg["rcp"][:], rcp_sb[:])
                    nc.sync.dma_start(dbg["q0"][:], qpair[0][:])
                    nc.sync.dma_start(dbg["k0"][:], kp[0][:])
                    nc.sync.dma_start(dbg["vaug"][:], v_aug[:])

            # ---- emission schedule: one flat attention stream ----
            wo = []
            wo_units = []
            for ch in range(NCH):
                w = const.tile([128, C], F16, name=f"wout{ch}")
                wo.append(w)

                def mk_wo(ch=ch, w=w):
                    def f():
                        nc.sync.dma_start(w[:],
                                          wout_d[ch * 128:(ch + 1) * 128, :])
                    return f
                wo_units.append(mk_wo())

            def qkts(p):
                # need-order: q/k nh0 halves first (used from t=0 of the
                # pair's first group), nh1 halves after (used from t=4).
                return [qkT_unit(p, qk, nh) for nh in range(2)
                        for qk in range(2)]

            with nc.named_scope("b0_proj"):
                # only the nh=0 halves up front; scores t=0..3 need just these.
                qkT_unit(0, 0, 0)()
                qkT_unit(0, 1, 0)()

            b0_nh0, b0_nh1 = out_units(0)
            b1_nh0, b1_nh1 = out_units(1)
            b0_all = b0_nh0 + b0_nh1

            groups = [(b, p, ic) for b in range(NB) for p in range(NPAIR)
                      for ic in range(2)]
            group_fill = {
                0: [qkT_unit(0, 0, 1), qkT_unit(0, 1, 1)] + qkts(1),
                2: qkts(2), 3: qkts(3),
                4: [xload_unit(1, ch) for ch in range(NCH)],
                5: wo_units,
                6: qkts(0),                      # batch-1 weights from here
                7: qkts(1), 9: qkts(2), 10: qkts(3),
                11: b0_all[0:3], 12: b0_all[3:6], 13: b0_all[6:8],
                15: b1_nh0,
            }
            # v(b1, 2u..2u+1) is legal only after attnv(b0,p3,ic1,u) (WAR on
            # v_aug) and before attnv(b1,p0,ic0,u) (RAW): pop 2 right after
            # group 7's attnv pair emissions.
            group_post = {7: deque(v_unit(t) for t in range(NT))}
            pre = deque(v_unit(t) for t in range(NT))   # batch-0 v, group 0

            nsteps = len(groups) * NT
            pending = deque()
            res_of = {}
            eb_of = {}
            with nc.named_scope("attn_stream"):
                for k in range(nsteps + 4):
                    g, t = divmod(k, NT)
                    if k < nsteps:
                        b, p, ic = groups[g]
                        if t == 0:
                            if g in group_fill:
                                filler.extend(group_fill[g])
                            res_of[g] = ps_acc.tile(
                                [128, N], F32, tag="res", bufs=1,
                                name=f"res{g}")
                        isl = slice(ic * 512, ic * 512 + 512)
                        js = slice(t * 128, (t + 1) * 128)
                        s_ps = ps_s.tile([128, N], F32, tag="big", name="s_ps")
                        nc.tensor.matmul(s_ps[:, 0:512], kp[p][0:64, js],
                                         qpair[p][0:64, isl],
                                         start=True, stop=True)
                        nc.tensor.matmul(s_ps[:, 512:1024], kp[p][64:128, js],
                                         qpair[p][64:128, isl],
                                         start=True, stop=True)
                        if t % 2 == 0:
                            eb_of[g, t // 2] = sbexp.tile(
                                [128, 2, N], F8E5, tag="exp", bufs=3,
                                name="exp_sb")
                        eb = eb_of[g, t // 2]
                        nc.scalar.activation(out=eb[:, t % 2, :], in_=s_ps[:],
                                             func=AF.Exp, bias=expbias[:],
                                             scale=SCALE)
                        if g == 0 and pre:
                            pre.popleft()()
                        if t % 2 == 1:
                            pending.append((g, t // 2))
                        elif t >= 2:
                            # t=0 is excluded: fillers must not be emitted
                            # before the cross-group norm pop at t=1.
                            inject(1)
                    if len(pending) == 2 or (k >= nsteps and pending):
                        pg, pu = pending.popleft()
                        pb, pp, pic = groups[pg]
                        peb = eb_of.pop((pg, pu))
                        if DEBUG and (pg, pu) == (0, 0):
                            nc.sync.dma_start(dbg["exp0"][:], peb[:])
                        for s in range(2):
                            nc.tensor.matmul(
                                res_of[pg][:, s * 512:(s + 1) * 512],
                                v_aug[:, pu, :, 2 * pp + s, :],
                                peb[:, :, s * 512:(s + 1) * 512],
                                perf_mode=DR,
                                start=(pu == 0), stop=(pu == NU - 1))
                        if pu == NU - 1:
                            norm_group(pb, pp, pic, res_of.pop(pg))
                        if pg in group_post and group_post[pg]:
                            group_post[pg].popleft()()
                            group_post[pg].popleft()()
                        else:
                            inject(1)

            with nc.named_scope("b1_out"):
                inject(len(filler))
                for u in b1_nh1:
                    u()
                if DEBUG:
                    for p in range(NPAIR):
                        nc.sync.dma_start(dbg["respair"][p],
                                          res_all_db[0][p][:])

    nc.finalize()
    return nc


_NC = None


def _get_nc():
    global _NC
    if _NC is None:
        _NC = build()
    return _NC


def _to_e4m3(a):
    return np.clip(a, -240, 240).astype(ml_dtypes.float8_e4m3fn)


def make_in_maps(x, W_qkv, b_qkv, W_out, b_out):
    x = np.ascontiguousarray(np.asarray(x, np.float32)).reshape(16, C, N)
    b_out = np.asarray(b_out, np.float32)
    xpb = np.ascontiguousarray(x + b_out[None, :, None]).astype(np.float16)
    w3 = np.asarray(W_qkv, np.float32).reshape(C, N_HEADS, 3, DK)
    w_qk = np.ascontiguousarray(
        np.stack([w3[:, :, 0], w3[:, :, 1]], axis=1).reshape(C, 2, NPAIR, 128))
    w_v = np.ascontiguousarray(w3[:, :, 2].reshape(C, C))
    # pair-interleave the contraction dim for DoubleRow: [cpair, 128, 2, out]
    w_v8 = _to_e4m3(w_v.reshape(NCP, 2, 128, C).transpose(0, 2, 1, 3))
    x8 = _to_e4m3(x.reshape(16, NCP, 2, 128, N).transpose(0, 1, 3, 2, 4))
    b3 = np.asarray(b_qkv, np.float32).reshape(N_HEADS, 3, DK)
    b_qk_t = np.ascontiguousarray(
        np.stack([b3[:, 0], b3[:, 1]], axis=0)
        .reshape(2, NPAIR, 128).transpose(2, 0, 1))
    b_v = np.ascontiguousarray(b3[:, 2].reshape(1, C))
    z8 = np.zeros((1, NU, 2, N_HEADS, VW), dtype=ml_dtypes.float8_e4m3fn)
    maps = []
    for core in range(NCORES):
        maps.append({
            "x": x[core * NB:(core + 1) * NB].astype(np.float16),
            "x8": np.ascontiguousarray(x8[core * NB:(core + 1) * NB]),
            "xpb": xpb[core * NB:(core + 1) * NB],
            "w_qk": w_qk.astype(np.float16),
            "b_qk_t": b_qk_t,
            "w_v8": np.ascontiguousarray(w_v8),
            "b_v": b_v,
            "w_out": np.asarray(W_out, np.float16),
            "z8": z8,
        })
    return maps


def run_on_hw(in_maps, **kwargs):
    nc = _get_nc()
    return bass_utils.run_bass_kernel_spmd(
        nc, in_maps, core_ids=list(range(NCORES)), **kwargs)


def kernel(x, W_qkv, b_qkv, W_out, b_out):
    res = run_on_hw(make_in_maps(x, W_qkv, b_qkv, W_out, b_out))
    y = np.concatenate([r["y"] for r in res.results], axis=0)  # (16, C, N)
    return y.reshape(16, C, 32, 32).astype(np.float32)


# revision 29
# speedup vs baseline: 1.1411x; 1.0073x over previous
"""Trainium2 Bass kernel for nn_AttentionBlock (B=16, C=512, H=W=32, 8 heads, d_k=64).

Sharding: data-parallel over batch; each of the 8 NeuronCores computes 2 batches.

Layout is fully transposed (channels on partitions) so no transposes are needed.
The attention phase is ACT(exp)-bound (16.8M exps/core @ 1 elem/lane/cycle,
1.2 GHz); everything else is scheduled into the PE/DVE slack under the exp
stream:

  qkT projection  : fp16 matmuls; qpair[p] = [q_{2p}; q_{2p+1}]^T stacked on
                    partition halves, kp[p] likewise (no zero padding).
  scores          : two K=64 matmuls per step (rows 0-63 / 64-127).
  exp             : ACT -> fp8-e5m2, shift -2.5 keeps exp in e5m2 range;
                    softmax-invariant. Output kappa-interleaved in
                    expbuf[128, 2, 1024] so pairs of j-tiles feed DoubleRow.
  attnv           : fp8 DoubleRow matmuls (2x): [1|0pad|v] e4m3 stationary
                    [128,2,128] x exp e5m2 moving [128,2,512] accumulating
                    res[128, 1024] over 4 j-tile pairs; row 0 = sumexp
                    (reciprocal_approx_fast needs base partition 0), v-dims
                    at rows 64-127 (64-partition ops need base 0/64).
  norm            : res evacuated PSUM->SBUF in one copy (frees the banks for
                    the next group), reciprocal_approx_fast, DMA partition-
                    broadcast via DRAM, 2 DVE mults -> res_pair e4m3.
  v projection    : fp8 DoubleRow matmuls (2x) over pair-interleaved
                    x8/wv8 (host-prepared).
  out projection  : fp16 matmuls (fp8 here costs too much accuracy) +
                    residual (x + b_out prefused, fp16); y stored fp16.

Schedule: one flat 128-step exp stream (16 groups of (batch, head-pair,
query-half) x 8 j-tiles); projection/output-projection/DMA work is interleaved
as PE filler so ACT never starves.
"""
from collections import deque

import numpy as np
import ml_dtypes

import concourse.bass as bass
from concourse import bacc
import concourse.mybir as mybir
import concourse.tile as tile
from concourse import bass_utils

F32 = mybir.dt.float32
F16 = mybir.dt.float16
F8E4 = mybir.dt.float8e4
F8E5 = mybir.dt.float8e5
AF = mybir.ActivationFunctionType
ALU = mybir.AluOpType
DR = mybir.MatmulPerfMode.DoubleRow

N_HEADS = 8
DK = 64
SCALE = DK ** -0.5
EXP_SHIFT = -2.5       # max scaled logit ~12.2 -> max exp arg ~9.7, e^9.7=16k
C = 512                # < e5m2 max 57344; per-query max weight >= 0.46
N = 1024               # tokens per batch (32*32)
NB = 2                 # batches per core
NCORES = 8
NCH = C // 128         # 4 contraction chunks
NCP = NCH // 2         # 2 DoubleRow chunk-pairs
NT = N // 128          # 8 token tiles
NU = NT // 2           # 4 j-tile pairs for DoubleRow attnv
NPAIR = N_HEADS // 2
VW = 128               # per-head v_aug extent (1 ones + 63 pad + 64 v dims)
DEBUG = False


def build():
    nc = bacc.Bacc(None, target_bir_lowering=False, num_swdge_queues=4)
    dbg = {}
    if DEBUG:
        dbg["q0"] = nc.dram_tensor("dbg_q0", (128, N), F16, kind="ExternalOutput")
        dbg["k0"] = nc.dram_tensor("dbg_k0", (128, N), F16, kind="ExternalOutput")
        dbg["exp0"] = nc.dram_tensor("dbg_exp0", (128, 2, N), F8E5, kind="ExternalOutput")
        dbg["vaug"] = nc.dram_tensor("dbg_vaug", (128, NU, 2, N_HEADS, VW), F8E4, kind="ExternalOutput")
        dbg["ressb"] = nc.dram_tensor("dbg_ressb", (128, N), F32, kind="ExternalOutput")
        dbg["rcp"] = nc.dram_tensor("dbg_rcp", (1, N), F32, kind="ExternalOutput")
        dbg["respair"] = nc.dram_tensor("dbg_respair", (NPAIR, 128, N), F16, kind="ExternalOutput")
    x_d = nc.dram_tensor("x", (NB, C, N), F16, kind="ExternalInput")
    x8_d = nc.dram_tensor("x8", (NB, NCP, 128, 2, N), F8E4, kind="ExternalInput")
    xpb_d = nc.dram_tensor("xpb", (NB, C, N), F16, kind="ExternalInput")
    wqk_d = nc.dram_tensor("w_qk", (C, 2, NPAIR, 128), F16, kind="ExternalInput")
    bqkt_d = nc.dram_tensor("b_qk_t", (128, 2, NPAIR), F32, kind="ExternalInput")
    wv_d = nc.dram_tensor("w_v8", (NCP, 128, 2, C), F8E4, kind="ExternalInput")
    bv_d = nc.dram_tensor("b_v", (1, C), F32, kind="ExternalInput")
    wout_d = nc.dram_tensor("w_out", (C, C), F16, kind="ExternalInput")
    z8_d = nc.dram_tensor("z8", (1, NU, 2, N_HEADS, VW), F8E4,
                          kind="ExternalInput")
    y_d = nc.dram_tensor("y", (NB, C, N), F16, kind="ExternalOutput")

    with tile.TileContext(nc) as tc:
        with (
            tc.tile_pool(name="const", bufs=1) as const,
            tc.tile_pool(name="persist", bufs=1) as persist,
            tc.tile_pool(name="sbwork", bufs=3) as sbwork,
            tc.tile_pool(name="sbexp", bufs=3) as sbexp,
            tc.tile_pool(name="ps_s", bufs=2, space="PSUM") as ps_s,
            tc.tile_pool(name="ps_acc", bufs=2, space="PSUM") as ps_acc,
            tc.tile_pool(name="dram", bufs=8, space="DRAM") as dram,
        ):
            # ---- weights / x loads (fp16 + fp8 pair-interleaved) ----
            x_r = [persist.tile([128, N], F16, name=f"xr{ch}")
                   for ch in range(NCH)]
            x8_r = [persist.tile([128, 2, N], F8E4, name=f"x8r{c}")
                    for c in range(NCP)]
            # v_aug zero-fill via DMA broadcast (a DVE/Pool memset of 1MB
            # would hog an engine for ~7us right when the head needs it)
            v_aug = persist.tile([128, NU, 2, N_HEADS, VW], F8E4)
            nc.scalar.dma_start(
                v_aug[:],
                z8_d[:].to_broadcast([128, NU, 2, N_HEADS, VW]))
            wqk = []
            xeng = [nc.sync, nc.scalar, nc.gpsimd, nc.gpsimd]
            weng = [nc.gpsimd, nc.gpsimd, nc.scalar, nc.sync]
            for ch in range(NCH):
                # x chunks first, spread over the 3 DMA queues (first qkT
                # needs all of them)
                xeng[ch].dma_start(x_r[ch][:], x_d[0, ch * 128:(ch + 1) * 128, :])
            for c in range(NCP):
                [nc.sync, nc.scalar][c].dma_start(x8_r[c][:], x8_d[0, c])
            for ch in range(NCH):
                w = const.tile([128, 2, NPAIR, 128], F16, name=f"wqk{ch}")
                weng[ch].dma_start(w[:], wqk_d[ch * 128:(ch + 1) * 128])
                wqk.append(w)
            wv8 = []
            for c in range(NCP):
                w2 = const.tile([128, 2, C], F8E4, name=f"wv8{c}")
                [nc.gpsimd, nc.sync][c].dma_start(w2[:], wv_d[c])
                wv8.append(w2)
            bqkt = const.tile([128, 2, NPAIR], F32)
            nc.sync.dma_start(bqkt[:], bqkt_d[:])
            bv_bc = const.tile([128, C], F32)   # b_v broadcast to all partitions
            nc.sync.dma_start(bv_bc[:], bv_d[:].to_broadcast([128, C]))

            # HAM warmup: dummy matmuls on memset data run during the initial
            # DMA wait so the real matmuls start at the full PE clock.
            warm = const.tile([128, 512], F16)
            nc.vector.memset(warm[:], 0.5)
            warm_ps = ps_acc.tile([128, 512], F32, tag="acc", name="warm_ps")
            for r in range(10):
                nc.tensor.matmul(warm_ps[:], warm[:, 0:128], warm[:],
                                 start=(r == 0), stop=(r == 9))
            # preload the exp table set during the DMA head (first real exp
            # would otherwise pay the ~2.7us ACT_TABLE_LOAD inside the stream)
            warm_exp = const.tile([128, 8], F16)
            nc.scalar.activation(out=warm_exp[:], in_=warm[:, 0:8],
                                 func=AF.Exp, scale=1.0)
            expbias = const.tile([128, 1], F32)
            nc.vector.memset(expbias[:], EXP_SHIFT)

            # ---- persistent per-batch buffers ----
            qpair = [persist.tile([128, N], F16, name=f"qpair{p}")
                     for p in range(NPAIR)]
            kp = [persist.tile([128, N], F16, name=f"kp{p}")
                  for p in range(NPAIR)]
            # ones row of v_aug (sumexp): [j, u, kappa, h, 0] = 1
            nc.vector.memset(v_aug[:, :, :, :, 0:1], 1.0)
            res_all_db = [[persist.tile([128, N], F16, name=f"resall{bb}_{p}")
                           for p in range(NPAIR)] for bb in range(NB)]

            # ---- work units (closures) for PE-filler interleaving ----
            xeng_mid = [nc.sync, nc.gpsimd, nc.gpsimd, nc.sync]

            def xload_unit(b, ch):
                def f():
                    # mid-stream: keep DMA issue off the busy Scalar queue
                    xeng_mid[ch].dma_start(x_r[ch][:],
                                           x_d[b, ch * 128:(ch + 1) * 128, :])
                    if ch % 2 == 0:
                        xeng_mid[ch + 1].dma_start(x8_r[ch // 2][:],
                                                   x8_d[b, ch // 2])
                return f

            def qkT_unit(p, qk, nh):
                def f():
                    nsl = slice(nh * 512, nh * 512 + 512)
                    ps = ps_acc.tile([128, 512], F32, tag="acc", name="qk_ps")
                    for ch in range(NCH):
                        nc.tensor.matmul(
                            ps[:], wqk[ch][:, qk, p, :], x_r[ch][:, nsl],
                            start=(ch == 0), stop=(ch == NCH - 1))
                    dst = qpair[p] if qk == 0 else kp[p]
                    nc.vector.tensor_scalar(
                        out=dst[:, nsl], in0=ps[:],
                        scalar1=bqkt[:, qk, p:p + 1], scalar2=None,
                        op0=ALU.add)
                return f

            def v_unit(t):
                def f():
                    ps = ps_acc.tile([128, 512], F32, tag="acc", name="v_ps")
                    for c in range(NCP):
                        nc.tensor.matmul(
                            ps[:], x8_r[c][:, :, t * 128:(t + 1) * 128],
                            wv8[c][:], perf_mode=DR,
                            start=(c == 0), stop=(c == NCP - 1))
                    nc.vector.tensor_add(
                        v_aug[:, t // 2, t % 2, :, 64:64 + DK],
                        ps[:].rearrange("p (h d) -> p h d", h=N_HEADS),
                        bv_bc[:].rearrange("p (h d) -> p h d", h=N_HEADS))
                return f

            def out_units(b):
                xres = {}
                units = []

                def mk(ct, nh):
                    def f():
                        csl = slice(ct * 128, (ct + 1) * 128)
                        nsl = slice(nh * 512, nh * 512 + 512)
                        if ct not in xres:
                            xr = sbwork.tile([128, N], F16, tag="xres", bufs=4,
                                             name=f"x_res{b}_{ct}")
                            nc.sync.dma_start(xr[:], xpb_d[b, csl, :])
                            xres[ct] = xr
                        ps = ps_acc.tile([128, 512], F32, tag="acc",
                                         name="out_ps")
                        for ch in range(NCH):
                            nc.tensor.matmul(
                                ps[:], wo[ch][:, csl],
                                res_all_db[b][ch][:, nsl],
                                start=(ch == 0), stop=(ch == NCH - 1))
                        out_sb = sbwork.tile([128, 512], F16, tag="out",
                                             name="out_sb")
                        nc.vector.tensor_add(out_sb[:], ps[:],
                                             xres[ct][:, nsl])
                        nc.sync.dma_start(y_d[b, csl, nsl], out_sb[:])
                    return f

                for nh in range(2):
                    for ct in range(NCH):
                        units.append(mk(ct, nh))
                return units[:NCH], units[NCH:]

            filler = deque()

            def inject(k=1):
                for _ in range(min(k, len(filler))):
                    filler.popleft()()

            def norm_group(b, p, ic, res_ps):
                # res_ps [128, 1024]: head 2p cols 0-511, head 2p+1 cols
                # 512-1023; row 0 = sumexp, rows 64-127 = v dims.  One
                # full-tile copy evacuates PSUM immediately (DVE cost is
                # free-dim-driven) — releases the res banks for the next
                # group ~4us earlier than normalizing out of PSUM would.
                # (Also: reciprocal_approx_fast reads garbage from PSUM.)
                isl = slice(ic * 512, ic * 512 + 512)
                res_sb = sbwork.tile([128, N], F32, tag="ressb", bufs=2,
                                     name="res_sb")
                nc.vector.tensor_copy(res_sb[:], res_ps[:])
                rcp_sb = sbwork.tile([1, N], F32, tag="sums", name="rcp_sb")
                nc.vector.reciprocal_approx_fast(out=rcp_sb[:],
                                                 in_=res_sb[0:1, :])
                rcp_dram = dram.tile([1, N], F32, tag="sumd", name="rcp_dram")
                nc.gpsimd.dma_start(rcp_dram[:], rcp_sb[:])
                # broadcast into rows 64-127 only, so the mult's two SBUF
                # inputs share base partition 64 (NCC_IBIR297)
                mult = sbwork.tile([128, N], F32, tag="mult", name="mult")
                nc.gpsimd.dma_start(mult[64:128, :],
                                    rcp_dram[:].to_broadcast([64, N]))
                for s in range(2):
                    nc.vector.tensor_mul(
                        res_all_db[b][p][s * 64:(s + 1) * 64, isl],
                        res_sb[64:64 + DK, s * 512:(s + 1) * 512],
                        mult[64:128, s * 512:(s + 1) * 512])
                if DEBUG and (b, p, ic) == (0, 0, 0):
                    nc.sync.dma_start(dbg["ressb"][:], res_sb[:])
                    nc.sync.dma_start(dbg["rcp"][:], rcp_sb[:])
                    nc.sync.dma_start(dbg["q0"][:], qpair[0][:])
                    nc.sync.dma_start(dbg["k0"][:], kp[0][:])
                    nc.sync.dma_start(dbg["vaug"][:], v_aug[:])

            # ---- emission schedule: one flat attention stream ----
            wo = []
            wo_units = []
            for ch in range(NCH):
                w = const.tile([128, C], F16, name=f"wout{ch}")
                wo.append(w)

                def mk_wo(ch=ch, w=w):
                    def f():
                        nc.sync.dma_start(w[:],
                                          wout_d[ch * 128:(ch + 1) * 128, :])
                    return f
                wo_units.append(mk_wo())

            def qkts(p):
                # need-order: q/k nh0 halves first (used from t=0 of the
                # pair's first group), nh1 halves after (used from t=4).
                return [qkT_unit(p, qk, nh) for nh in range(2)
                        for qk in range(2)]

            with nc.named_scope("b0_proj"):
                # only the nh=0 halves up front; scores t=0..3 need just these.
                qkT_unit(0, 0, 0)()
                qkT_unit(0, 1, 0)()

            b0_nh0, b0_nh1 = out_units(0)
            b1_nh0, b1_nh1 = out_units(1)
            b0_all = b0_nh0 + b0_nh1

            groups = [(b, p, ic) for b in range(NB) for p in range(NPAIR)
                      for ic in range(2)]
            group_fill = {
                0: [qkT_unit(0, 0, 1), qkT_unit(0, 1, 1)],
                1: qkts(1),             # g0 is already loaded with v-units
                2: qkts(2), 3: qkts(3),
                4: [xload_unit(1, ch) for ch in range(NCH)],
                5: wo_units,
                6: qkts(0),                      # batch-1 weights from here
                7: qkts(1), 9: qkts(2), 10: qkts(3),
                11: b0_all[0:3], 12: b0_all[3:6], 13: b0_all[6:8],
                15: b1_nh0,
            }
            # v(b1, 2u..2u+1) is legal only after attnv(b0,p3,ic1,u) (WAR on
            # v_aug) and before attnv(b1,p0,ic0,u) (RAW): pop 2 right after
            # group 7's attnv pair emissions.
            group_post = {7: deque(v_unit(t) for t in range(NT))}
            pre = deque(v_unit(t) for t in range(NT))   # batch-0 v, group 0

            nsteps = len(groups) * NT
            pending = deque()
            res_of = {}
            eb_of = {}
            with nc.named_scope("attn_stream"):
                for k in range(nsteps + 4):
                    g, t = divmod(k, NT)
                    if k < nsteps:
                        b, p, ic = groups[g]
                        if t == 0:
                            if g in group_fill:
                                filler.extend(group_fill[g])
                            res_of[g] = ps_acc.tile(
                                [128, N], F32, tag="res", bufs=1,
                                name=f"res{g}")
                        isl = slice(ic * 512, ic * 512 + 512)
                        js = slice(t * 128, (t + 1) * 128)
                        s_ps = ps_s.tile([128, N], F32, tag="big", name="s_ps")
                        nc.tensor.matmul(s_ps[:, 0:512], kp[p][0:64, js],
                                         qpair[p][0:64, isl],
                                         start=True, stop=True)
                        nc.tensor.matmul(s_ps[:, 512:1024], kp[p][64:128, js],
                                         qpair[p][64:128, isl],
                                         start=True, stop=True)
                        if t % 2 == 0:
                            eb_of[g, t // 2] = sbexp.tile(
                                [128, 2, N], F8E5, tag="exp", bufs=4,
                                name="exp_sb")
                        eb = eb_of[g, t // 2]
                        nc.scalar.activation(out=eb[:, t % 2, :], in_=s_ps[:],
                                             func=AF.Exp, bias=expbias[:],
                                             scale=SCALE)
                        if g == 0 and pre:
                            pre.popleft()()
                        if t % 2 == 1:
                            pending.append((g, t // 2))
                        elif t >= 2:
                            # t=0 is excluded: fillers must not be emitted
                            # before the cross-group norm pop at t=1.
                            inject(1)
                    if len(pending) == 2 or (k >= nsteps and pending):
                        pg, pu = pending.popleft()
                        pb, pp, pic = groups[pg]
                        peb = eb_of.pop((pg, pu))
                        if DEBUG and (pg, pu) == (0, 0):
                            nc.sync.dma_start(dbg["exp0"][:], peb[:])
                        for s in range(2):
                            nc.tensor.matmul(
                                res_of[pg][:, s * 512:(s + 1) * 512],
                                v_aug[:, pu, :, 2 * pp + s, :],
                                peb[:, :, s * 512:(s + 1) * 512],
                                perf_mode=DR,
                                start=(pu == 0), stop=(pu == NU - 1))
                        if pu == NU - 1:
                            norm_group(pb, pp, pic, res_of.pop(pg))
                        if pg in group_post and group_post[pg]:
                            group_post[pg].popleft()()
                            group_post[pg].popleft()()
                        else:
                            inject(1)

            with nc.named_scope("b1_out"):
                inject(len(filler))
                for u in b1_nh1:
                    u()
                if DEBUG:
                    for p in range(NPAIR):
                        nc.sync.dma_start(dbg["respair"][p],
                                          res_all_db[0][p][:])

    nc.finalize()
    return nc


_NC = None


def _get_nc():
    global _NC
    if _NC is None:
        _NC = build()
    return _NC


def _to_e4m3(a):
    return np.clip(a, -240, 240).astype(ml_dtypes.float8_e4m3fn)


def make_in_maps(x, W_qkv, b_qkv, W_out, b_out):
    x = np.ascontiguousarray(np.asarray(x, np.float32)).reshape(16, C, N)
    b_out = np.asarray(b_out, np.float32)
    xpb = np.ascontiguousarray(x + b_out[None, :, None]).astype(np.float16)
    w3 = np.asarray(W_qkv, np.float32).reshape(C, N_HEADS, 3, DK)
    w_qk = np.ascontiguousarray(
        np.stack([w3[:, :, 0], w3[:, :, 1]], axis=1).reshape(C, 2, NPAIR, 128))
    w_v = np.ascontiguousarray(w3[:, :, 2].reshape(C, C))
    # pair-interleave the contraction dim for DoubleRow: [cpair, 128, 2, out]
    w_v8 = _to_e4m3(w_v.reshape(NCP, 2, 128, C).transpose(0, 2, 1, 3))
    x8 = _to_e4m3(x.reshape(16, NCP, 2, 128, N).transpose(0, 1, 3, 2, 4))
    b3 = np.asarray(b_qkv, np.float32).reshape(N_HEADS, 3, DK)
    b_qk_t = np.ascontiguousarray(
        np.stack([b3[:, 0], b3[:, 1]], axis=0)
        .reshape(2, NPAIR, 128).transpose(2, 0, 1))
    b_v = np.ascontiguousarray(b3[:, 2].reshape(1, C))
    z8 = np.zeros((1, NU, 2, N_HEADS, VW), dtype=ml_dtypes.float8_e4m3fn)
    maps = []
    for core in range(NCORES):
        maps.append({
            "x": x[core * NB:(core + 1) * NB].astype(np.float16),
            "x8": np.ascontiguousarray(x8[core * NB:(core + 1) * NB]),
            "xpb": xpb[core * NB:(core + 1) * NB],
            "w_qk": w_qk.astype(np.float16),
            "b_qk_t": b_qk_t,
            "w_v8": np.ascontiguousarray(w_v8),
            "b_v": b_v,
            "w_out": np.asarray(W_out, np.float16),
            "z8": z8,
        })
    return maps


def run_on_hw(in_maps, **kwargs):
    nc = _get_nc()
    return bass_utils.run_bass_kernel_spmd(
        nc, in_maps, core_ids=list(range(NCORES)), **kwargs)


def kernel(x, W_qkv, b_qkv, W_out, b_out):
    res = run_on_hw(make_in_maps(x, W_qkv, b_qkv, W_out, b_out))
    y = np.concatenate([r["y"] for r in res.results], axis=0)  # (16, C, N)
    return y.reshape(16, C, 32, 32).astype(np.float32)
